# revision 2
# baseline (speedup 1.0000x reference)
"""GAT-style masked self-attention (B=4, N=4096, D=128) on 8 trn2 NeuronCores.

reference:
    scores = X @ X^T / sqrt(D)            [B, N, N]
    masked = where(adj > 0, scores, -1e12)
    attn   = softmax(masked, axis=2)
    out    = attn @ X                     [B, N, D]

Sharding: 8 cores <- (batch b, row-half h); each core handles 2048 rows
of one batch element against all 4096 keys. No collectives.

Device algorithm (per core), orientation "S^T" (keys on partitions).
X is host-prescaled by sqrt(ALPHA) so the score matmul's PSUM value is
directly the fast-exp fixed-point argument psS = raw_dot*ALPHA.

Mask handling, two balanced paths:

  k-tiles 24..31 (NK_F=8), DVE fast-exp with the mask baked in:
      u16 = convert(psS + madd)   [one tensor_tensor add]
    madd (host int16) = unmasked ? C : -32768.  Unmasked: Schraudolph
    fast exp -- the u16 bit pattern read as fp16 is exp(raw_dot*SCALE-8);
    masked: the sum is negative and the u16 convert clamps to 0.  The
    host clamps diagonal entries so the sum stays below the fp16 inf
    region.  Result feeds the AV matmul directly (bitcast fp16).

  k-tiles 0..23: mask shipped as uint8 {0,1}, SWDGE cast-DMA to bf16
    {0,1} in SBUF.  ACT evicts psS with exp fused (bfloat16), then the
    masked prob ptm = p * m_u is computed as a bf16 tensor_tensor mult
    (2x DVE mode), with one of every six k-tiles routed to GpSimd to
    take load off DVE.

  AV matmul with the denominator fused via an appended ones-column:
      psO[rc] (+)= w_k.T @ [X_k | 1] over k in (24..31, 0..23)
  psO (raw numerator | denominator) is evicted by DVE copies and
  DMA'd out; the host performs the final divide.

  Row blocks [256,512,512,512,256] are software-pipelined: block i runs
  scores/exp/mask while block i-1 runs its AV matmuls; AV matmuls are
  emitted first within each group so PE covers the eviction drain.
  ~24 junk matmuls at t=0 warm the PE HAM clock gate during the initial
  DMA wait; junk LDWEIGHTS at drain start keep it open.
"""

import math
import sys

sys.path.insert(0, "/opt/trn_rl_repo")

import numpy as np

B, N, D = 4, 4096, 128
R = N // 2            # rows per core
NK = N // 128         # 32 key tiles
NK_M = 24             # key tiles masked multiplicatively (0..NK_M-1)
NK_F = NK - NK_M      # fast-exp key tiles (NK_M..31)
SCALE = 1.0 / math.sqrt(D)
EXP_BIAS = -8.0

# fast-exp: exp(z) ~ f16_bits(round(z*1024/ln2 + 15360 - ADJ))
LOG2_SC = (1 << 10) / math.log(2.0)     # 1477.3196
ALPHA = SCALE * LOG2_SC                 # psS = raw_dot * ALPHA
SQ_ALPHA = math.sqrt(ALPHA)             # host pre-scale per score operand
FEXP_ADJ = 50.0
FEXP_C = float(round((15 << 10) + EXP_BIAS * LOG2_SC - FEXP_ADJ))
ACT_SCALE = SCALE / ALPHA               # makes ACT see raw_dot*SCALE

# row blocks (offset, size): small edge blocks shorten fill and drain
BLOCKS = [(0, 256), (256, 512), (768, 512), (1280, 512), (1792, 256)]
NB = len(BLOCKS)
NRC = R // 128                          # 16 row tiles of 128

# mask-mul spans of 6 k-tiles; evict groups never cross span boundaries.
SPAN = 6
MUL_SPANS = [(s, SPAN) for s in range(0, NK_M, SPAN)]
GPS_PER_SPAN = 1                        # trailing k-tiles of each span -> gpsimd

K_ORDER = list(range(NK_M, NK)) + list(range(NK_M))
K_FIRST, K_LAST = K_ORDER[0], K_ORDER[-1]

N_WARM = 24                             # PE warm-up matmuls (FD=128)

_CACHE = {}


def _groups(bs):
    """Evict groups: fexp tiles first, then ACT tiles; width 3 for 512-row
    blocks (PSUM limit), 6 for 256-row blocks; never cross span bounds."""
    w = 3 if bs == 512 else 6
    gs = []
    for k0 in range(NK_M, NK, w):
        gs.append(list(range(k0, min(k0 + w, NK))))
    for k0 in range(0, NK_M, w):
        gs.append(list(range(k0, k0 + w)))
    return gs


def _build_nc(cfg):
    from concourse import bacc
    import concourse.mybir as mybir
    from concourse.tile import TileContext

    dt = mybir.dt

    nc = bacc.Bacc(None, target_bir_lowering=False)

    xt_d = nc.dram_tensor("xt", [D, N], dt.float16, kind="ExternalInput")
    xtr_d = nc.dram_tensor("xtr", [D, R], dt.float16, kind="ExternalInput")
    # [128, k, d] with xaug_p[p, k, :] = [X | 1][k*128+p, :], host-packed
    xaug_d = nc.dram_tensor("xaug", [128, NK * (D + 1)], dt.bfloat16,
                            kind="ExternalInput")
    # {0,1} mask for key tiles 0..NK_M-1, block-major columns
    m2_d = nc.dram_tensor("m2", [128, NK_M * R], dt.uint8,
                          kind="ExternalInput")
    # additive fast-exp mask for key tiles NK_M..31, block-major columns
    madd_d = nc.dram_tensor("madd", [128, NK_F * R], dt.int16,
                            kind="ExternalInput")
    # [numerator | denominator] per row tile, block-major columns
    o_d = nc.dram_tensor("o", [128, NRC * (D + 1)], dt.float32,
                         kind="ExternalOutput")

    with TileContext(nc) as tc:
        with (
            tc.tile_pool(name="singles", bufs=1) as singles,
            tc.tile_pool(name="madd", bufs=2) as madd_pool,
            tc.tile_pool(name="mu", bufs=2) as mu_pool,
            tc.tile_pool(name="ptm", bufs=2) as ptm_pool,
            tc.tile_pool(name="pe", bufs=1) as pe_pool,
            tc.tile_pool(name="pef", bufs=2) as pef_pool,
            tc.tile_pool(name="outs", bufs=4) as out_pool,
            tc.tile_pool(name="small", bufs=4) as small_pool,
            tc.tile_pool(name="psS", bufs=2, space="PSUM") as psS_pool,
            tc.tile_pool(name="psO", bufs=2, space="PSUM") as psO_pool,
        ):
            # --- PE warm-up: open the HAM clock gate during the DMA wait
            junk = singles.tile([128, 128], dt.float16)
            nc.vector.memset(junk[:], 0.0)
            psW = psS_pool.tile([128, 3, 512], mybir.dt.float32,
                                tag="psS", name="psS_warm")
            for _ in range(N_WARM):
                nc.tensor.matmul(psW[:, 0, 0:128], lhsT=junk[:],
                                 rhs=junk[:], start=True, stop=True)

            ebias = singles.tile([128, 1], mybir.dt.float32)
            nc.vector.memset(ebias[:], EXP_BIAS)
            # warm the exp table while the init DMAs stream in
            warm = small_pool.tile([128, 1], mybir.dt.float32, tag="warm")
            nc.vector.memset(warm[:], 0.0)
            warm2 = small_pool.tile([128, 1], mybir.dt.float32, tag="warm")
            nc.scalar.activation(
                warm2[:], warm[:], mybir.ActivationFunctionType.Exp, scale=1.0
            )

            xt_sb = singles.tile([D, N], dt.float16)
            xtr_sb = singles.tile([D, R], dt.float16)
            xaug_sb = singles.tile([128, NK, D + 1], dt.bfloat16)
            mu_tiles = {}
            madd_tiles = {}

            def fetch_mask(ph):
                off_p, bs_p = BLOCKS[ph]
                t = madd_pool.tile([128, NK_F, bs_p], dt.int16, tag="madd",
                                   name=f"madd_{ph}")
                madd_tiles[ph] = t
                c0 = NK_F * off_p
                nc.gpsimd.dma_start(
                    out=t[:], in_=madd_d[:, c0:c0 + NK_F * bs_p])
                t = mu_pool.tile([128, NK_M, bs_p], dt.bfloat16, tag="mu",
                                 name=f"mu_{ph}")
                mu_tiles[ph] = t
                c0 = NK_M * off_p
                nc.gpsimd.dma_start(
                    out=t[:], in_=m2_d[:, c0:c0 + NK_M * bs_p])

            # init DMAs in first-consumption order.  sync ring:
            nc.sync.dma_start(out=xtr_sb[:, 0:256], in_=xtr_d[:, 0:256])
            nc.sync.dma_start(out=xt_sb[:, NK_M * 128:4096],
                              in_=xt_d[:, NK_M * 128:4096])
            nc.sync.dma_start(out=xt_sb[:, 0:1536], in_=xt_d[:, 0:1536])
            nc.sync.dma_start(out=xt_sb[:, 1536:NK_M * 128],
                              in_=xt_d[:, 1536:NK_M * 128])
            nc.sync.dma_start(out=xaug_sb[:], in_=xaug_d[:, :])
            nc.sync.dma_start(out=xtr_sb[:, 256:1024], in_=xtr_d[:, 256:1024])
            nc.sync.dma_start(out=xtr_sb[:, 1024:2048],
                              in_=xtr_d[:, 1024:2048])
            # gpsimd ring: block0 madd (first evict group) + mask
            fetch_mask(0)

            ptm_prev = None
            pef_prev = None
            bs_prev = None
            off_prev = None

            def emit_av(psO, k, rc):
                if k >= NK_M:
                    lhsT = pef_prev[:, k - NK_M, rc * 128:(rc + 1) * 128].bitcast(
                        dt.float16
                    )
                else:
                    lhsT = ptm_prev[:, k, rc * 128:(rc + 1) * 128]
                nc.tensor.matmul(
                    psO[rc // 2][:, rc % 2, :],
                    lhsT=lhsT,
                    rhs=xaug_sb[:, k, :],
                    start=(k == K_FIRST),
                    stop=(k == K_LAST),
                )

            def evict_psO(psO, g, ph_out_c0):
                # raw [num | denom] rows: PSUM -> SBUF -> DRAM; host divides
                o_sb = out_pool.tile([128, 2, D + 1], mybir.dt.float32,
                                     tag="o", name=f"o_{ph_out_c0}_{g}")
                nc.vector.tensor_copy(o_sb[:], psO[g][:])
                c = ph_out_c0 + 2 * g * (D + 1)
                nc.sync.dma_start(out=o_d[:, c:c + 2 * (D + 1)], in_=o_sb[:])

            span_of = {}
            for k0, nkk in MUL_SPANS:
                for k in range(k0, k0 + nkk):
                    span_of[k] = k0

            for phase in range(NB + 1):
                ptm_cur = None
                pef_cur = None
                psO = None
                if phase < NB:
                    off, bs = BLOCKS[phase]
                    m_u = mu_tiles[phase]
                    if phase + 1 < NB:
                        fetch_mask(phase + 1)
                    ptm_cur = ptm_pool.tile([128, NK_M, bs], dt.bfloat16,
                                            tag="ptm", name=f"ptm_{phase}")
                    pef_cur = pef_pool.tile([128, NK_F, bs], dt.uint16,
                                            tag="pef", name=f"pef_{phase}")
                    madd_t = madd_tiles.get(phase)
                if phase >= 1:
                    # pairs of [128, 129] accumulators packed per PSUM bank
                    psO = [
                        psO_pool.tile(
                            [128, 2, D + 1], mybir.dt.float32,
                            tag="psO", name=f"psO_{phase}_{g}",
                        )
                        for g in range(bs_prev // 256)
                    ]
                    out_c0 = (off_prev // 128) * (D + 1)

                if phase == NB:
                    # drain: junk LDWEIGHTS keep the clock gate open while
                    # the last block's mults finish, then rc-major AV bursts
                    for _ in range(24):
                        nc.tensor.ldweights(junk[:])
                    for rc in range(bs_prev // 128):
                        for k in K_ORDER:
                            emit_av(psO, k, rc)
                        if rc % 2 == 1:
                            evict_psO(psO, rc // 2, out_c0)
                    break

                groups = _groups(bs)
                pe_span = {
                    k0: pe_pool.tile([128, nkk, bs], dt.bfloat16,
                                     tag=f"pe{k0}", name=f"pe_{phase}_{k0}")
                    for k0, nkk in MUL_SPANS
                }
                done = set()
                muls_emitted = set()
                # AV ops rc-major: the two accumulation streams sharing a
                # PSUM bank run sequentially, never interleaved within a bank
                av_ops = []
                if phase >= 1:
                    for rc in range(bs_prev // 128):
                        for k in K_ORDER:
                            av_ops.append((rc, k))
                ng = len(groups)
                av_pos = 0
                for gi, gks in enumerate(groups):
                    # AV matmuls for the previous block first: PE has work
                    # while the eviction drains this group's scores
                    n_av = (len(av_ops) * (gi + 1)) // ng - (len(av_ops) * gi) // ng
                    for _ in range(n_av):
                        rc, k = av_ops[av_pos]
                        av_pos += 1
                        emit_av(psO, k, rc)
                    gw = len(gks)
                    ps = psS_pool.tile([128, 3 if bs == 512 else 6, bs],
                                       mybir.dt.float32,
                                       tag="psS", name=f"psS_{phase}_{gi}")
                    for j, k in enumerate(gks):
                        nc.tensor.matmul(
                            ps[:, j, :],
                            lhsT=xt_sb[:, k * 128:(k + 1) * 128],
                            rhs=xtr_sb[:, off:off + bs],
                            start=True,
                            stop=True,
                        )
                    k0g = gks[0]
                    if k0g >= NK_M:
                        f0 = k0g - NK_M
                        nc.vector.tensor_tensor(
                            pef_cur[:, f0:f0 + gw, :],
                            ps[:, 0:gw, :],
                            madd_t[:, f0:f0 + gw, :],
                            mybir.AluOpType.add,
                        )
                    else:
                        k0s = span_of[k0g]
                        i0 = k0g - k0s
                        nc.scalar.activation(
                            pe_span[k0s][:, i0:i0 + gw, :],
                            ps[:, 0:gw, :],
                            mybir.ActivationFunctionType.Exp,
                            bias=ebias[:],
                            scale=ACT_SCALE,
                        )
                    done.update(gks)
                    # masked probs for spans whose evictions completed;
                    # trailing tiles of each span go to gpsimd
                    for k0s, nkk in MUL_SPANS:
                        if k0s in muls_emitted:
                            continue
                        if all((k0s + t) in done for t in range(nkk)):
                            muls_emitted.add(k0s)
                            nd = nkk - GPS_PER_SPAN
                            nc.vector.tensor_tensor(
                                ptm_cur[:, k0s:k0s + nd, :],
                                pe_span[k0s][:, 0:nd, :],
                                m_u[:, k0s:k0s + nd, :],
                                mybir.AluOpType.mult,
                            )
                            if GPS_PER_SPAN:
                                nc.gpsimd.tensor_tensor(
                                    ptm_cur[:, k0s + nd:k0s + nkk, :],
                                    pe_span[k0s][:, nd:nkk, :],
                                    m_u[:, k0s + nd:k0s + nkk, :],
                                    mybir.AluOpType.mult,
                                )
                if phase >= 1:
                    for g in range(bs_prev // 256):
                        evict_psO(psO, g, out_c0)
                ptm_prev = ptm_cur
                pef_prev = pef_cur
                bs_prev = bs
                off_prev = off
    nc.finalize()
    return nc


def _get_nc():
    if "nc" not in _CACHE:
        _CACHE["nc"] = _build_nc(None)
    return _CACHE["nc"]


def make_in_maps(input, adj):
    """Host-side shard/layout prep: one input map per core."""
    import ml_dtypes

    input = np.asarray(input, dtype=np.float32)
    adj = np.asarray(adj)

    in_maps = []
    for core in range(8):
        b, h = core // 2, core % 2
        xb = input[b]                                    # [N, D]
        xs = (xb.T * SQ_ALPHA).astype(np.float16)        # pre-scaled scores
        xt = np.ascontiguousarray(xs)
        xtr = np.ascontiguousarray(xs[:, h * R:(h + 1) * R])
        xaug = np.concatenate([xb, np.ones((N, 1), np.float32)], axis=1)
        xaug = xaug.astype(ml_dtypes.bfloat16)
        xaug_p = np.ascontiguousarray(
            xaug.reshape(NK, 128, D + 1).transpose(1, 0, 2)
        ).reshape(128, NK * (D + 1))
        s = adj[b][h * R:(h + 1) * R, :] > 0             # [R rows, N cols]

        # m2[p, kt, r] {0,1} for tiles 0..NK_M-1, then block-major columns
        sm = s[:, : NK_M * 128].reshape(R, NK_M, 128)
        m2f = sm.transpose(2, 1, 0).astype(np.uint8)     # [128, NK_M, R]
        m2 = np.concatenate(
            [m2f[:, :, off:off + bs].reshape(128, NK_M * bs)
             for off, bs in BLOCKS], axis=1)
        m2 = np.ascontiguousarray(m2)

        # madd[p, kt, r] for keys NK_M*128.. : additive fast-exp mask
        m3 = s[:, NK_M * 128:]                           # [R, NK_F*128]
        madd = np.where(m3, FEXP_C, -32768.0)            # [R, cols]
        if h == 1:
            # diagonal keys (global row == key) in the fexp range; clamp
            # so psS_diag + madd stays below the fp16 inf bit region
            xs64 = xs.astype(np.float64)
            g = np.arange(NK_M * 128, 4096)              # global fexp keys
            r_idx = g - R                                # local row
            ps_diag = (xs64[:, g] * xs64[:, g]).sum(axis=0)
            cap = 31500.0 - ps_diag
            col = g - NK_M * 128
            cur = madd[r_idx, col]
            madd[r_idx, col] = np.where(
                m3[r_idx, col], np.minimum(cur, cap), cur
            )
        maddf = np.round(
            madd.reshape(R, NK_F, 128).transpose(2, 1, 0)
        ).astype(np.int16)                               # [128, NK_F, R]
        maddb = np.concatenate(
            [maddf[:, :, off:off + bs].reshape(128, NK_F * bs)
             for off, bs in BLOCKS], axis=1)
        maddb = np.ascontiguousarray(maddb)
        in_maps.append({
            "xt": xt, "xtr": xtr, "xaug": xaug_p,
            "m2": m2, "madd": maddb,
        })
    return in_maps


def run_device(in_maps, trace=False, trace_cores=None):
    import concourse.bass_utils as bass_utils

    if trace:
        bass_utils.upload_artifacts = lambda tmpdir: ""  # no bucket in sandbox
    nc = _get_nc()
    return bass_utils.run_bass_kernel_spmd(
        nc, in_maps, list(range(8)), trace=trace, trace_cores=trace_cores
    )


def unshard(res):
    """Assemble + normalize the per-core raw [num | denom] outputs."""
    out = np.empty((B, N, D), dtype=np.float32)
    for core in range(8):
        b, h = core // 2, core % 2
        o = np.asarray(res.results[core]["o"], dtype=np.float32)
        o = o.reshape(128, NRC, D + 1).transpose(1, 0, 2).reshape(R, D + 1)
        out[b, h * R:(h + 1) * R, :] = o[:, :D] / o[:, D:D + 1]
    return out


def kernel(input, adj):
    res = run_device(make_in_maps(input, adj))
    return unshard(res)


# revision 7
# speedup vs baseline: 1.0073x; 1.0073x over previous
"""GAT-style masked self-attention (B=4, N=4096, D=128) on 8 trn2 NeuronCores.

reference:
    scores = X @ X^T / sqrt(D)            [B, N, N]
    masked = where(adj > 0, scores, -1e12)
    attn   = softmax(masked, axis=2)
    out    = attn @ X                     [B, N, D]

Sharding: 8 cores <- (batch b, row-half h); each core handles 2048 rows
of one batch element against all 4096 keys. No collectives.

Device algorithm (per core), orientation "S^T" (keys on partitions).
X is host-prescaled by sqrt(ALPHA) so the score matmul's PSUM value is
directly the fast-exp fixed-point argument psS = raw_dot*ALPHA.

Mask handling, two balanced paths:

  k-tiles 24..31 (NK_F=8), DVE fast-exp with the mask baked in:
      u16 = convert(psS + madd)   [one tensor_tensor add]
    madd (host int16) = unmasked ? C : -32768.  Unmasked: Schraudolph
    fast exp -- the u16 bit pattern read as fp16 is exp(raw_dot*SCALE-8);
    masked: the sum is negative and the u16 convert clamps to 0.  The
    host clamps diagonal entries so the sum stays below the fp16 inf
    region.  Result feeds the AV matmul directly (bitcast fp16).

  k-tiles 0..23: mask shipped as uint8 {0,1}, SWDGE cast-DMA to bf16
    {0,1} in SBUF.  ACT evicts psS with exp fused (bfloat16), then the
    masked prob ptm = p * m_u is computed as a bf16 tensor_tensor mult
    (2x DVE mode), with one of every six k-tiles routed to GpSimd to
    take load off DVE.

  AV matmul with the denominator fused via an appended ones-column:
      psO[rc] (+)= w_k.T @ [X_k | 1] over k in (24..31, 0..23)
  psO (raw numerator | denominator) is evicted by DVE copies and
  DMA'd out; the host performs the final divide.

  Row blocks [256,512,512,512,256] are software-pipelined: block i runs
  scores/exp/mask while block i-1 runs its AV matmuls; AV matmuls are
  emitted first within each group so PE covers the eviction drain.
  ~24 junk matmuls at t=0 warm the PE HAM clock gate during the initial
  DMA wait; junk LDWEIGHTS at drain start keep it open.
"""

import math
import sys

sys.path.insert(0, "/opt/trn_rl_repo")

import numpy as np

B, N, D = 4, 4096, 128
R = N // 2            # rows per core
NK = N // 128         # 32 key tiles
NK_M = 24             # key tiles masked multiplicatively (0..NK_M-1)
NK_F = NK - NK_M      # fast-exp key tiles (NK_M..31)
SCALE = 1.0 / math.sqrt(D)
EXP_BIAS = -8.0

# fast-exp: exp(z) ~ f16_bits(round(z*1024/ln2 + 15360 - ADJ))
LOG2_SC = (1 << 10) / math.log(2.0)     # 1477.3196
ALPHA = SCALE * LOG2_SC                 # psS = raw_dot * ALPHA
SQ_ALPHA = math.sqrt(ALPHA)             # host pre-scale per score operand
FEXP_ADJ = 50.0
FEXP_C = float(round((15 << 10) + EXP_BIAS * LOG2_SC - FEXP_ADJ))
ACT_SCALE = SCALE / ALPHA               # makes ACT see raw_dot*SCALE

# row blocks (offset, size): small edge blocks shorten fill and drain
BLOCKS = [(0, 256), (256, 512), (768, 512), (1280, 512), (1792, 256)]
NB = len(BLOCKS)
NRC = R // 128                          # 16 row tiles of 128

# mask-mul spans of 6 k-tiles; evict groups never cross span boundaries.
SPAN = 6
MUL_SPANS = [(s, SPAN) for s in range(0, NK_M, SPAN)]
GPS_PER_SPAN = 1                        # trailing k-tiles of each span -> gpsimd

K_ORDER = list(range(NK_M, NK)) + list(range(NK_M))
K_FIRST, K_LAST = K_ORDER[0], K_ORDER[-1]

N_WARM = 24                             # PE warm-up matmuls (FD=128)

_CACHE = {}


def _groups(bs):
    """Evict groups: fexp tiles first, then ACT tiles; width 3 for 512-row
    blocks (PSUM limit), 6 for 256-row blocks; never cross span bounds."""
    w = 3 if bs == 512 else 6
    gs = []
    for k0 in range(NK_M, NK, w):
        gs.append(list(range(k0, min(k0 + w, NK))))
    for k0 in range(0, NK_M, w):
        gs.append(list(range(k0, k0 + w)))
    return gs


def _build_nc(cfg):
    from concourse import bacc
    import concourse.mybir as mybir
    from concourse.tile import TileContext

    dt = mybir.dt

    nc = bacc.Bacc(None, target_bir_lowering=False)

    xt_d = nc.dram_tensor("xt", [D, N], dt.float16, kind="ExternalInput")
    xtr_d = nc.dram_tensor("xtr", [D, R], dt.float16, kind="ExternalInput")
    # [128, k, d] with xaug_p[p, k, :] = [X | 1][k*128+p, :], host-packed
    xaug_d = nc.dram_tensor("xaug", [128, NK * (D + 1)], dt.bfloat16,
                            kind="ExternalInput")
    # {0,1} mask for key tiles 0..NK_M-1, block-major columns
    m2_d = nc.dram_tensor("m2", [128, NK_M * R], dt.uint8,
                          kind="ExternalInput")
    # additive fast-exp mask for key tiles NK_M..31, block-major columns
    madd_d = nc.dram_tensor("madd", [128, NK_F * R], dt.int16,
                            kind="ExternalInput")
    # [numerator | denominator] per row tile, block-major columns
    o_d = nc.dram_tensor("o", [128, NRC * (D + 1)], dt.float32,
                         kind="ExternalOutput")

    with TileContext(nc) as tc:
        with (
            tc.tile_pool(name="singles", bufs=1) as singles,
            tc.tile_pool(name="madd", bufs=3) as madd_pool,
            tc.tile_pool(name="mu", bufs=4) as mu_pool,
            tc.tile_pool(name="pe", bufs=1) as pe_pool,
            tc.tile_pool(name="pef", bufs=2) as pef_pool,
            tc.tile_pool(name="outs", bufs=4) as out_pool,
            tc.tile_pool(name="small", bufs=4) as small_pool,
            tc.tile_pool(name="psS", bufs=2, space="PSUM") as psS_pool,
            tc.tile_pool(name="psO", bufs=2, space="PSUM") as psO_pool,
        ):
            # --- PE warm-up: open the HAM clock gate during the DMA wait
            junk = singles.tile([128, 128], dt.float16)
            nc.vector.memset(junk[:], 0.0)
            psW = psS_pool.tile([128, 3, 512], mybir.dt.float32,
                                tag="psS", name="psS_warm")
            for _ in range(N_WARM):
                nc.tensor.matmul(psW[:, 0, 0:128], lhsT=junk[:],
                                 rhs=junk[:], start=True, stop=True)

            ebias = singles.tile([128, 1], mybir.dt.float32)
            nc.vector.memset(ebias[:], EXP_BIAS)
            # warm the exp table while the init DMAs stream in
            warm = small_pool.tile([128, 1], mybir.dt.float32, tag="warm")
            nc.vector.memset(warm[:], 0.0)
            warm2 = small_pool.tile([128, 1], mybir.dt.float32, tag="warm")
            nc.scalar.activation(
                warm2[:], warm[:], mybir.ActivationFunctionType.Exp, scale=1.0
            )

            xt_sb = singles.tile([D, N], dt.float16)
            xtr_sb = singles.tile([D, R], dt.float16)
            xaug_sb = singles.tile([128, NK, D + 1], dt.bfloat16)
            mu_tiles = {}
            madd_tiles = {}

            def fetch_madd(ph):
                # madd needs no cast -> HWDGE sync ring (cheap triggers)
                off_p, bs_p = BLOCKS[ph]
                t = madd_pool.tile([128, NK_F, bs_p], dt.int16, tag="madd",
                                   name=f"madd_{ph}")
                madd_tiles[ph] = t
                c0 = NK_F * off_p
                nc.sync.dma_start(
                    out=t[:], in_=madd_d[:, c0:c0 + NK_F * bs_p])

            def fetch_m2(ph):
                off_p, bs_p = BLOCKS[ph]
                t = mu_pool.tile([128, NK_M, bs_p], dt.bfloat16, tag="mu",
                                 name=f"mu_{ph}")
                mu_tiles[ph] = t
                c0 = NK_M * off_p
                nc.gpsimd.dma_start(
                    out=t[:], in_=m2_d[:, c0:c0 + NK_M * bs_p])

            # init DMAs in first-consumption order.  sync ring:
            nc.sync.dma_start(out=xtr_sb[:, 0:256], in_=xtr_d[:, 0:256])
            nc.sync.dma_start(out=xt_sb[:, NK_M * 128:4096],
                              in_=xt_d[:, NK_M * 128:4096])
            fetch_madd(0)
            nc.sync.dma_start(out=xt_sb[:, 0:1536], in_=xt_d[:, 0:1536])
            nc.sync.dma_start(out=xt_sb[:, 1536:NK_M * 128],
                              in_=xt_d[:, 1536:NK_M * 128])
            fetch_madd(1)
            nc.sync.dma_start(out=xtr_sb[:, 256:1024], in_=xtr_d[:, 256:1024])
            nc.sync.dma_start(out=xaug_sb[:], in_=xaug_d[:, :])
            nc.sync.dma_start(out=xtr_sb[:, 1024:2048],
                              in_=xtr_d[:, 1024:2048])
            # gpsimd ring: masks for the first two blocks
            fetch_m2(0)
            fetch_m2(1)

            ptm_prev = None
            pef_prev = None
            bs_prev = None
            off_prev = None

            def emit_av(psO, k, rc):
                if k >= NK_M:
                    lhsT = pef_prev[:, k - NK_M, rc * 128:(rc + 1) * 128].bitcast(
                        dt.float16
                    )
                else:
                    lhsT = ptm_prev[:, k, rc * 128:(rc + 1) * 128]
                nc.tensor.matmul(
                    psO[rc // 2][:, rc % 2, :],
                    lhsT=lhsT,
                    rhs=xaug_sb[:, k, :],
                    start=(k == K_FIRST),
                    stop=(k == K_LAST),
                )

            def evict_psO(psO, g, ph_out_c0):
                # raw [num | denom] rows: PSUM -> SBUF -> DRAM; host divides
                o_sb = out_pool.tile([128, 2, D + 1], mybir.dt.float32,
                                     tag="o", name=f"o_{ph_out_c0}_{g}")
                nc.vector.tensor_copy(o_sb[:], psO[g][:])
                c = ph_out_c0 + 2 * g * (D + 1)
                nc.sync.dma_start(out=o_d[:, c:c + 2 * (D + 1)], in_=o_sb[:])

            span_of = {}
            for k0, nkk in MUL_SPANS:
                for k in range(k0, k0 + nkk):
                    span_of[k] = k0

            for phase in range(NB + 1):
                ptm_cur = None
                pef_cur = None
                psO = None
                if phase < NB:
                    off, bs = BLOCKS[phase]
                    m_u = mu_tiles[phase]
                    if phase + 2 < NB:
                        fetch_madd(phase + 2)
                        fetch_m2(phase + 2)
                    # masked probs overwrite the {0,1} mask in place
                    ptm_cur = m_u
                    pef_cur = pef_pool.tile([128, NK_F, bs], dt.uint16,
                                            tag="pef", name=f"pef_{phase}")
                    madd_t = madd_tiles.get(phase)
                if phase >= 1:
                    # pairs of [128, 129] accumulators packed per PSUM bank
                    psO = [
                        psO_pool.tile(
                            [128, 2, D + 1], mybir.dt.float32,
                            tag="psO", name=f"psO_{phase}_{g}",
                        )
                        for g in range(bs_prev // 256)
                    ]
                    out_c0 = (off_prev // 128) * (D + 1)

                if phase == NB:
                    # drain: junk LDWEIGHTS keep the clock gate open while
                    # the last block's mults finish, then rc-major AV bursts
                    for _ in range(24):
                        nc.tensor.ldweights(junk[:])
                    for rc in range(bs_prev // 128):
                        for k in K_ORDER:
                            emit_av(psO, k, rc)
                        if rc % 2 == 1:
                            evict_psO(psO, rc // 2, out_c0)
                    break

                groups = _groups(bs)
                pe_span = {
                    k0: pe_pool.tile([128, nkk, bs], dt.bfloat16,
                                     tag=f"pe{k0}", name=f"pe_{phase}_{k0}")
                    for k0, nkk in MUL_SPANS
                }
                done = set()
                muls_emitted = set()
                # AV ops rc-major: the two accumulation streams sharing a
                # PSUM bank run sequentially, never interleaved within a bank
                av_ops = []
                if phase >= 1:
                    for rc in range(bs_prev // 128):
                        for k in K_ORDER:
                            av_ops.append((rc, k))
                ng = len(groups)
                av_pos = 0
                for gi, gks in enumerate(groups):
                    # AV matmuls for the previous block first: PE has work
                    # while the eviction drains this group's scores
                    n_av = (len(av_ops) * (gi + 1)) // ng - (len(av_ops) * gi) // ng
                    for _ in range(n_av):
                        rc, k = av_ops[av_pos]
                        av_pos += 1
                        emit_av(psO, k, rc)
                    gw = len(gks)
                    ps = psS_pool.tile([128, 3 if bs == 512 else 6, bs],
                                       mybir.dt.float32,
                                       tag="psS", name=f"psS_{phase}_{gi}")
                    for j, k in enumerate(gks):
                        nc.tensor.matmul(
                            ps[:, j, :],
                            lhsT=xt_sb[:, k * 128:(k + 1) * 128],
                            rhs=xtr_sb[:, off:off + bs],
                            start=True,
                            stop=True,
                        )
                    k0g = gks[0]
                    if k0g >= NK_M:
                        f0 = k0g - NK_M
                        nc.vector.tensor_tensor(
                            pef_cur[:, f0:f0 + gw, :],
                            ps[:, 0:gw, :],
                            madd_t[:, f0:f0 + gw, :],
                            mybir.AluOpType.add,
                        )
                    else:
                        k0s = span_of[k0g]
                        i0 = k0g - k0s
                        nc.scalar.activation(
                            pe_span[k0s][:, i0:i0 + gw, :],
                            ps[:, 0:gw, :],
                            mybir.ActivationFunctionType.Exp,
                            bias=ebias[:],
                            scale=ACT_SCALE,
                        )
                    done.update(gks)
                    # masked probs for spans whose evictions completed;
                    # trailing tiles of each span go to gpsimd
                    for k0s, nkk in MUL_SPANS:
                        if k0s in muls_emitted:
                            continue
                        if all((k0s + t) in done for t in range(nkk)):
                            muls_emitted.add(k0s)
                            nd = nkk - GPS_PER_SPAN
                            nc.vector.tensor_tensor(
                                m_u[:, k0s:k0s + nd, :],
                                pe_span[k0s][:, 0:nd, :],
                                m_u[:, k0s:k0s + nd, :],
                                mybir.AluOpType.mult,
                            )
                            if GPS_PER_SPAN:
                                nc.gpsimd.tensor_tensor(
                                    m_u[:, k0s + nd:k0s + nkk, :],
                                    pe_span[k0s][:, nd:nkk, :],
                                    m_u[:, k0s + nd:k0s + nkk, :],
                                    mybir.AluOpType.mult,
                                )
                if phase >= 1:
                    for g in range(bs_prev // 256):
                        evict_psO(psO, g, out_c0)
                ptm_prev = ptm_cur
                pef_prev = pef_cur
                bs_prev = bs
                off_prev = off
    nc.finalize()
    return nc


def _get_nc():
    if "nc" not in _CACHE:
        _CACHE["nc"] = _build_nc(None)
    return _CACHE["nc"]


def make_in_maps(input, adj):
    """Host-side shard/layout prep: one input map per core."""
    import ml_dtypes

    input = np.asarray(input, dtype=np.float32)
    adj = np.asarray(adj)

    in_maps = []
    for core in range(8):
        b, h = core // 2, core % 2
        xb = input[b]                                    # [N, D]
        xs = (xb.T * SQ_ALPHA).astype(np.float16)        # pre-scaled scores
        xt = np.ascontiguousarray(xs)
        xtr = np.ascontiguousarray(xs[:, h * R:(h + 1) * R])
        xaug = np.concatenate([xb, np.ones((N, 1), np.float32)], axis=1)
        xaug = xaug.astype(ml_dtypes.bfloat16)
        xaug_p = np.ascontiguousarray(
            xaug.reshape(NK, 128, D + 1).transpose(1, 0, 2)
        ).reshape(128, NK * (D + 1))
        s = adj[b][h * R:(h + 1) * R, :] > 0             # [R rows, N cols]

        # m2[p, kt, r] {0,1} for tiles 0..NK_M-1, then block-major columns
        sm = s[:, : NK_M * 128].reshape(R, NK_M, 128)
        m2f = sm.transpose(2, 1, 0).astype(np.uint8)     # [128, NK_M, R]
        m2 = np.concatenate(
            [m2f[:, :, off:off + bs].reshape(128, NK_M * bs)
             for off, bs in BLOCKS], axis=1)
        m2 = np.ascontiguousarray(m2)

        # madd[p, kt, r] for keys NK_M*128.. : additive fast-exp mask
        m3 = s[:, NK_M * 128:]                           # [R, NK_F*128]
        madd = np.where(m3, FEXP_C, -32768.0)            # [R, cols]
        if h == 1:
            # diagonal keys (global row == key) in the fexp range; clamp
            # so psS_diag + madd stays below the fp16 inf bit region
            xs64 = xs.astype(np.float64)
            g = np.arange(NK_M * 128, 4096)              # global fexp keys
            r_idx = g - R                                # local row
            ps_diag = (xs64[:, g] * xs64[:, g]).sum(axis=0)
            cap = 31500.0 - ps_diag
            col = g - NK_M * 128
            cur = madd[r_idx, col]
            madd[r_idx, col] = np.where(
                m3[r_idx, col], np.minimum(cur, cap), cur
            )
        maddf = np.round(
            madd.reshape(R, NK_F, 128).transpose(2, 1, 0)
        ).astype(np.int16)                               # [128, NK_F, R]
        maddb = np.concatenate(
            [maddf[:, :, off:off + bs].reshape(128, NK_F * bs)
             for off, bs in BLOCKS], axis=1)
        maddb = np.ascontiguousarray(maddb)
        in_maps.append({
            "xt": xt, "xtr": xtr, "xaug": xaug_p,
            "m2": m2, "madd": maddb,
        })
    return in_maps


def run_device(in_maps, trace=False, trace_cores=None):
    import concourse.bass_utils as bass_utils

    if trace:
        bass_utils.upload_artifacts = lambda tmpdir: ""  # no bucket in sandbox
    nc = _get_nc()
    return bass_utils.run_bass_kernel_spmd(
        nc, in_maps, list(range(8)), trace=trace, trace_cores=trace_cores
    )


def unshard(res):
    """Assemble + normalize the per-core raw [num | denom] outputs."""
    out = np.empty((B, N, D), dtype=np.float32)
    for core in range(8):
        b, h = core // 2, core % 2
        o = np.asarray(res.results[core]["o"], dtype=np.float32)
        o = o.reshape(128, NRC, D + 1).transpose(1, 0, 2).reshape(R, D + 1)
        out[b, h * R:(h + 1) * R, :] = o[:, :D] / o[:, D:D + 1]
    return out


def kernel(input, adj):
    res = run_device(make_in_maps(input, adj))
    return unshard(res)
